# revision 20
# baseline (speedup 1.0000x reference)
"""DYAN encoder (FISTA sparse coding) as a Bass/Tile kernel on 8 trn2 NeuronCores.

Algorithm notes
---------------
reference computes, with D [T=10, K=645] (normalized dictionary), Y = x[0] [10, P]:
    A   = I - D^T D / L,  c = D^T Y / L,  lam = 0.1 / L
    y_0 = x_0 = 0
    for j in 0..99:   (the early-stop never triggers for this data)
        w      = A y_j + c = y_j + (1/L) D^T (Y - D y_j)
        x_{j+1} = softshrink(w, lam)
        y_{j+1} = (1+tt_j) x_{j+1} - tt_j x_j
Since A is I minus a rank-10 term, each iteration only needs thin matmuls:
    u_j = Y - D x_j                    [10, P]   (PE, contraction 645+10)
    w   = (1/L) D^T ((1+tt) u_j - tt u_{j-1}) - tt x_{j-1} + (1+tt) x_j
    x_{j+1} = shrink(w)
Engine mapping (v3):
  * u MMs: bf16 — stationary s_d holds -D scattered twice (rows 0:10 / 32:42 of
    a 128-wide padded output); the moving operand is a zero-copy bf16 truncated
    view of the fp32 x tiles (hi 2 bytes of each fp32). Y rides chunk 5: its x
    tile is extended to [15, P] with rows 5:15 = Y and the chunk-5 stationary
    gets +identity rows, so no separate Y matmul.
  * A/B scaled copies of u ride ScalarE (fp32 psum -> fp16 ab tiles).
  * w MMs: fp16 — merged rank-20 stationary wab; moving = ab.
  * the -tt x_{j-1} term is an exact fp32 ScalarE/VectorE PRE-WRITE into the w
    psum bank; the w matmuls then accumulate onto it with start=False. This
    works because the PSUM has_written bits are set once by the start=True
    groups of iterations 0/1 and never cleared afterwards, so PE accumulates
    onto engine-written data. Replaces 12 fp32r id-matmuls + weight loads per
    iteration (the dominant PE cost and precision bottleneck of v0-v2).
  * x_{j+1} = shrink(w_psum + (1+tt) x_j) is one fused custom DVE op per chunk
    (in1 = fp32 x_j keeps the momentum path exact).

Sharding: pure data parallel over the pixel dim P (8192 -> 8 x 1024).
"""

import os
import numpy as np

T = 10
NDICT = 161
K = 4 * NDICT + 1          # 645
P_FULL = 8192
N_CORES = 8
P = P_FULL // N_CORES      # 1024
NH = 512                   # psum-bank half width (fp32)
CH = [128, 128, 128, 128, 128, 5]   # K split into partition chunks
XR = [128, 128, 128, 128, 128, 15]  # x-tile rows (chunk 5 carries Y in rows 5:15)
OFF = [0, 128, 256, 384, 512, 640]
NITER = 100
LAMBD = 0.1

# engine per chunk for the -tt*x_{j-1} psum pre-writes
PRE_ENG = ["act", "act", "act", "act", "act", "dve"]

_cache = {}


# --------------------------------------------------------------------------- #
# custom DVE ops
# --------------------------------------------------------------------------- #
def _register_dve_op(name, spec):
    import concourse.dve_ops as dve_ops_mod
    from concourse.dve_spec import lower, _has_src1
    from concourse.dve_uop import DveOpSpec

    for o in dve_ops_mod.OPS:
        if o.name == name:
            return o
    row = dve_ops_mod._CUSTOM_DVE_ROW_BASE + len(dve_ops_mod.OPS)
    assert row < 0x20, "DVE opcode rows exhausted"
    shas = {}
    for ver in ("v3", "v4"):
        s = DveOpSpec(name=name, opcode=row, uops=lower(spec, ver=ver),
                      rd1_en=_has_src1(spec))
        shas[ver] = s.sha(ver)
    op = dve_ops_mod.DveOp(name, spec, subdim=False, uops_sha=shas)
    dve_ops_mod.OPS.append(op)
    dve_ops_mod._SUB_OPCODE_FOR_NAME[name] = row
    dve_ops_mod.CUSTOM_DVE_SPECS[name] = spec
    return op


def _get_shrink_op():
    """out = v - clamp(v, -s1, s1) with v = in0 + s0*in1  (softshrink fused
    with the momentum-weighted x add; in0 comes straight from PSUM)."""
    from concourse.dve_spec import Spec, Src0, Src1, C0, C1, C2, maxx, minn

    v = Src0 + C0 * Src1
    body = v - minn(maxx(v, C2), C1)

    def _ref(in0, in1, s0, s1, imm2):
        v = in0.astype(np.float32) + np.float32(s0) * in1.astype(np.float32)
        return v - np.minimum(np.maximum(v, np.float32(imm2)), np.float32(s1))

    return _register_dve_op("FISTA_SHRINK_ANT", Spec(body=body, reference=_ref))


def _get_shrink0_op():
    """out = in0 - clamp(in0, -s1, s1)  (softshrink only; used at iteration 0
    where x_0 = 0 so there is no momentum term)."""
    from concourse.dve_spec import Spec, Src0, C0, C1, maxx, minn

    body = Src0 - minn(maxx(Src0, C0), C1)

    def _ref(in0, in1, s0, s1, imm2):
        v = in0.astype(np.float32)
        return v - np.minimum(np.maximum(v, np.float32(s0)), np.float32(s1))

    return _register_dve_op("FISTA_SHRINK0_ANT", Spec(body=body, reference=_ref))


# --------------------------------------------------------------------------- #
# host-side precompute
# --------------------------------------------------------------------------- #
def _build_dictionary(rr, theta, t):
    i = np.arange(t, dtype=np.float64)[:, None]
    rr = rr.astype(np.float64)
    theta = theta.astype(np.float64)
    rp = rr[None, :] ** i
    sgn = np.where(np.arange(t)[:, None] % 2 == 0, 1.0, -1.0)
    c = np.cos(i * theta[None, :])
    s = np.sin(i * theta[None, :])
    ones = np.ones((t, 1))
    dic = np.concatenate([ones, rp * c, sgn * rp * c, rp * s, sgn * rp * s], axis=1)
    g = np.linalg.norm(dic, axis=0)
    g = np.where(g == 0, np.sqrt(t), g)
    return dic / g


def _momentum_coeffs(n_iter):
    ts = []
    t = 1.0
    for _ in range(n_iter):
        t_new = (1.0 + np.sqrt(1.0 + 4.0 * t * t)) / 2.0
        ts.append((t - 1.0) / t_new)
        t = t_new
    return np.asarray(ts, dtype=np.float32)


# --------------------------------------------------------------------------- #
# device module
# --------------------------------------------------------------------------- #
def _build_module(lam, linv, tts):
    import concourse.bacc as bacc
    import concourse.mybir as mybir
    import concourse.tile as tile

    F32 = mybir.dt.float32
    F32R = mybir.dt.float32r
    F16 = mybir.dt.float16
    BF16 = mybir.dt.bfloat16
    shrink_op = _get_shrink_op()
    shrink0_op = _get_shrink0_op()

    nc = bacc.Bacc("TRN2", target_bir_lowering=False, debug=False)

    y_d = nc.dram_tensor("y_in", [T, P], F32, kind="ExternalInput").ap()
    sd_d = nc.dram_tensor("s_d", [K + T, 128], BF16, kind="ExternalInput").ap()
    wab_d = nc.dram_tensor("w_ab", [42, 768], F16, kind="ExternalInput").ap()
    z_d = nc.dram_tensor("zeros", [22, P], F16, kind="ExternalInput").ap()
    z32_d = nc.dram_tensor("zeros32", [CH[5], P], F32, kind="ExternalInput").ap()
    out_d = nc.dram_tensor("out", [K, P], F32, kind="ExternalOutput").ap()

    # per-iteration scalars (fp32-exact python floats)
    tt_prev = [0.0] + [float(tts[j]) for j in range(NITER - 1)]
    lam_f = float(np.float32(lam))
    linv_f = float(np.float32(linv))

    with tile.TileContext(nc) as tc:
        with (
            tc.tile_pool(name="const", bufs=1) as const,
            tc.tile_pool(name="state", bufs=1) as state,
            tc.tile_pool(name="upool", bufs=2, space="PSUM") as upool,
            tc.tile_pool(name="wpool", bufs=3, space="PSUM") as wpool,
        ):
            wab_t = const.tile([42, 768], F16, tag="wab", name="wab_t")
            sd_t = [const.tile([XR[c], 128], BF16, tag=f"sd{c}", name=f"sd_t{c}")
                    for c in range(6)]

            nc.sync.dma_start(out=wab_t[:], in_=wab_d[:])
            so = [0, 128, 256, 384, 512, 640, 655]
            for c in range(6):
                nc.sync.dma_start(out=sd_t[c][:], in_=sd_d[so[c]:so[c + 1], :])

            xt = [[state.tile([XR[c], P], F32, tag=f"x{g}_{c}", name=f"x{g}_{c}")
                   for c in range(6)] for g in range(3)]
            ab_ts = [state.tile([42, P], F16, tag=f"AB{p}", name=f"ab_t{p}")
                     for p in range(2)]
            # rows 10..31 are dead contraction lanes of the merged matmul:
            # must be finite (stationary rows there are zero)
            for p in range(2):
                nc.sync.dma_start(out=ab_ts[p][10:32, :], in_=z_d[:])
            # chunk 5 carries Y in rows 5:15 of every generation; rows 0:5 of
            # the j=0 generation must be zero (x_0 = 0)
            for g in range(3):
                nc.sync.dma_start(out=xt[g][5][5:15, :], in_=y_d[:])
            nc.sync.dma_start(out=xt[0][5][0:5, :], in_=z32_d[:])

            def bfview(ap):
                return ap.bitcast(BF16)[:, 1::2]

            # Iteration specialization:
            #   j=0: x_0 = x_{-1} = 0 -> u_0 = Y via the chunk-5 matmul alone,
            #        no pre-write, plain shrink, w group start=True.
            #   j=1: tt_prev = 0 -> no pre-write, w start=True; B_0 is zero
            #        data but the b-term matmul contraction (42 rows) is used.
            for j in range(NITER):
                ttp = tt_prev[j]
                gm1, g0, g1 = (j + 2) % 3, j % 3, (j + 1) % 3
                ab_cur = ab_ts[j % 2]
                ab_next = ab_ts[(j + 1) % 2]
                a_scale = float(np.float32((1.0 + ttp) * linv_f))
                b_scale = float(np.float32(-float(tts[j]) * linv_f))
                has_ux = j >= 1        # x_j nonzero
                has_id = ttp != 0.0    # j >= 2
                has_m2old = j >= 1     # B_{j-1} exists

                # u = Y - D x_j  (replicated at partition offsets 0/32; Y rides
                # the chunk-5 stationary's identity rows). Per-half psum tiles
                # (1 bank each, double-buffered) so iteration j+1's accumulation
                # can start while iteration j's copies still drain.
                cs_u = list(range(6)) if has_ux else [5]
                u_ps = [upool.tile([128, NH], F32, tag="u", name=f"u_ps{h}")
                        for h in (0, 1)]
                # halves interleaved: each chunk's shrink unblocks two
                # back-to-back matmuls, halving PE idle during the u phase
                for ci, c in enumerate(cs_u):
                    for h in (0, 1):
                        sl = slice(NH * h, NH * (h + 1))
                        nc.tensor.matmul(u_ps[h][:], sd_t[c][:],
                                         bfview(xt[g0][c][:])[:, sl],
                                         start=(ci == 0), stop=(c == 5))
                # scaled copies: A_j = (1+tt)/L u_j (used now),
                #                B_j = -tts[j]/L u_j (used next iteration)
                for h in (0, 1):
                    sl = slice(NH * h, NH * (h + 1))
                    nc.scalar.mul(ab_cur[0:T, sl], u_ps[h][0:T, :], a_scale)
                    if j < NITER - 1:
                        nc.scalar.mul(ab_next[32:42, sl], u_ps[h][32:42, :],
                                      b_scale)

                for wv in (0, 1):
                    cs = [3 * wv, 3 * wv + 1, 3 * wv + 2]
                    wt = {c: wpool.tile([CH[c], P], F32, tag="w", name=f"w{c}")
                          for c in cs}
                    # identity part: w = -tt * x_{j-1} as an exact fp32
                    # engine pre-write; the matmuls accumulate onto it
                    # (has_written bits persist from the j<2 start=True groups)
                    if has_id:
                        for c in cs:
                            src = xt[gm1][c][0:CH[c], :]
                            if PRE_ENG[c] == "act":
                                nc.scalar.mul(wt[c][:], src, float(np.float32(-ttp)))
                            else:
                                nc.vector.tensor_scalar_mul(wt[c][:], src,
                                                            float(np.float32(-ttp)))
                    # rank-10+10 part in one matmul: w += [D;0;D]^T [A;junk;B]
                    kc = 42 if has_m2old else T
                    for h in (0, 1):
                        sl = slice(NH * h, NH * (h + 1))
                        for c in cs:
                            nc.tensor.matmul(
                                wt[c][:, sl],
                                wab_t[0:kc, 128 * c:128 * c + CH[c]],
                                ab_cur[0:kc, sl],
                                start=not has_id, stop=True,
                                skip_group_check=has_id)
                    # x_{j+1} = shrink(w + (1+tt) x_j)
                    for c in cs:
                        if has_ux:
                            nc.vector._custom_dve(
                                shrink_op, out=xt[g1][c][0:CH[c], :],
                                in0=wt[c][:],
                                in1=xt[g0][c][0:CH[c], :],
                                s0=float(np.float32(1.0 + ttp)), s1=lam_f,
                                imm2=-lam_f)
                        else:
                            nc.vector._custom_dve(
                                shrink0_op, out=xt[g1][c][0:CH[c], :],
                                in0=wt[c][:],
                                s0=-lam_f, s1=lam_f)
                        if j == NITER - 1:
                            nc.sync.dma_start(
                                out=out_d[OFF[c]:OFF[c] + CH[c], :],
                                in_=xt[g1][c][0:CH[c], :])

    nc.compile()
    return nc


# --------------------------------------------------------------------------- #
# entry point
# --------------------------------------------------------------------------- #
def _prepare(x, Drr, Dtheta, t):
    import ml_dtypes

    x = np.asarray(x, dtype=np.float32)
    d64 = _build_dictionary(np.asarray(Drr), np.asarray(Dtheta), t)
    dtd = d64.T @ d64
    lspec = np.linalg.norm(dtd, ord=2)
    linv = 1.0 / lspec
    lam = LAMBD * linv
    d32 = d64.astype(np.float32)
    tts = _momentum_coeffs(NITER)

    # u = Y - D x is produced replicated at partition offsets 0 and 32 (the
    # 0-copy feeds the A scaled-copy, the 32-copy feeds the B scaled-copy).
    # Stationary free dims padded to 128 so FWL (fast weight load) triggers.
    # Rows K:K+T are the chunk-5 identity rows that inject Y.
    s_d = np.zeros((K + T, 128), dtype=np.float32)
    for r in (0, 1):
        s_d[:K, 32 * r:32 * r + T] = -d32.T
        s_d[K + np.arange(T), 32 * r + np.arange(T)] = 1.0
    # merged rank-20 stationary: rows 0..9 multiply A, rows 32..41 multiply B
    w_ab = np.zeros((42, 768), dtype=np.float32)
    for c in range(6):
        w_ab[0:T, 128 * c:128 * c + CH[c]] = d32[:, OFF[c]:OFF[c] + CH[c]]
        w_ab[32:42, 128 * c:128 * c + CH[c]] = d32[:, OFF[c]:OFF[c] + CH[c]]
    zeros = np.zeros((22, P), dtype=np.float16)
    zeros32 = np.zeros((CH[5], P), dtype=np.float32)
    return (x, lam, linv, tts, s_d.astype(ml_dtypes.bfloat16),
            w_ab.astype(np.float16), zeros, zeros32)


def run(x, Drr, Dtheta, T_in, trace=False):
    from concourse.bass_utils import run_bass_kernel_spmd

    t = int(np.asarray(T_in))
    assert t == T
    x, lam, linv, tts, s_d, w_ab, zeros, zeros32 = _prepare(x, Drr, Dtheta, t)

    key = ("mod", float(np.float32(lam)), float(np.float32(linv)))
    if key not in _cache:
        _cache[key] = _build_module(lam, linv, tts)
    nc = _cache[key]

    in_maps = []
    for core in range(N_CORES):
        in_maps.append({
            "y_in": np.ascontiguousarray(x[0, :, core * P:(core + 1) * P]),
            "s_d": s_d,
            "w_ab": w_ab,
            "zeros": zeros,
            "zeros32": zeros32,
        })
    res = run_bass_kernel_spmd(nc, in_maps, list(range(N_CORES)), trace=trace)
    out = np.concatenate([res.results[c]["out"] for c in range(N_CORES)], axis=1)
    return out[None, :, :].astype(np.float32), res


def kernel(x, Drr, Dtheta, T, **kw):
    out, _ = run(x, Drr, Dtheta, T, trace=bool(os.environ.get("FISTA_TRACE")))
    return out


# revision 26
# speedup vs baseline: 1.1392x; 1.1392x over previous
"""DYAN encoder (FISTA sparse coding) as a Bass/Tile kernel on 8 trn2 NeuronCores.

Algorithm notes
---------------
reference computes, with D [T=10, K=645] (normalized dictionary), Y = x[0] [10, P]:
    A   = I - D^T D / L,  c = D^T Y / L,  lam = 0.1 / L
    y_0 = x_0 = 0
    for j in 0..99:   (the early-stop never triggers for this data)
        w      = A y_j + c = y_j + (1/L) D^T (Y - D y_j)
        x_{j+1} = softshrink(w, lam)
        y_{j+1} = (1+tt_j) x_{j+1} - tt_j x_j
Since A is I minus a rank-10 term, each iteration only needs thin matmuls:
    u_j = Y - D x_j                    [10, P]   (PE, contraction 645+10)
    w   = (1/L) D^T ((1+tt) u_j - tt u_{j-1}) - tt x_{j-1} + (1+tt) x_j
    x_{j+1} = shrink(w)
Engine mapping (v3):
  * u MMs: bf16 — stationary s_d holds -D scattered twice (rows 0:10 / 32:42 of
    a 128-wide padded output); the moving operand is a zero-copy bf16 truncated
    view of the fp32 x tiles (hi 2 bytes of each fp32). Y rides chunk 5: its x
    tile is extended to [15, P] with rows 5:15 = Y and the chunk-5 stationary
    gets +identity rows, so no separate Y matmul.
  * A/B scaled copies of u ride ScalarE (fp32 psum -> fp16 ab tiles).
  * w MMs: fp16 — merged rank-20 stationary wab; moving = ab.
  * the -tt x_{j-1} term is an exact fp32 ScalarE/VectorE PRE-WRITE into the w
    psum bank; the w matmuls then accumulate onto it with start=False. This
    works because the PSUM has_written bits are set once by the start=True
    groups of iterations 0/1 and never cleared afterwards, so PE accumulates
    onto engine-written data. Replaces 12 fp32r id-matmuls + weight loads per
    iteration (the dominant PE cost and precision bottleneck of v0-v2).
  * x_{j+1} = shrink(w_psum + (1+tt) x_j) is one fused custom DVE op per chunk
    (in1 = fp32 x_j keeps the momentum path exact).

Sharding: pure data parallel over the pixel dim P (8192 -> 8 x 1024).
"""

import os
import numpy as np

T = 10
NDICT = 161
K = 4 * NDICT + 1          # 645
P_FULL = 8192
N_CORES = 8
P = P_FULL // N_CORES      # 1024
NH = 512                   # psum-bank half width (fp32)
CH = [128, 128, 128, 128, 128, 5]   # K split into partition chunks
XR = [128, 128, 128, 128, 128, 15]  # x-tile rows (chunk 5 carries Y in rows 5:15)
OFF = [0, 128, 256, 384, 512, 640]
NITER = 100
LAMBD = 0.1

# engine per chunk for the -tt*x_{j-1} psum pre-writes
PRE_ENG = ["act", "act", "act", "act", "act", "dve"]

_cache = {}


# --------------------------------------------------------------------------- #
# custom DVE ops
# --------------------------------------------------------------------------- #
def _register_dve_op(name, spec):
    import concourse.dve_ops as dve_ops_mod
    from concourse.dve_spec import lower, _has_src1
    from concourse.dve_uop import DveOpSpec

    for o in dve_ops_mod.OPS:
        if o.name == name:
            return o
    row = dve_ops_mod._CUSTOM_DVE_ROW_BASE + len(dve_ops_mod.OPS)
    assert row < 0x20, "DVE opcode rows exhausted"
    shas = {}
    for ver in ("v3", "v4"):
        s = DveOpSpec(name=name, opcode=row, uops=lower(spec, ver=ver),
                      rd1_en=_has_src1(spec))
        shas[ver] = s.sha(ver)
    op = dve_ops_mod.DveOp(name, spec, subdim=False, uops_sha=shas)
    dve_ops_mod.OPS.append(op)
    dve_ops_mod._SUB_OPCODE_FOR_NAME[name] = row
    dve_ops_mod.CUSTOM_DVE_SPECS[name] = spec
    return op


def _get_shrink_op():
    """out = v - clamp(v, -s1, s1) with v = in0 + s0*in1  (softshrink fused
    with the momentum-weighted x add; in0 comes straight from PSUM)."""
    from concourse.dve_spec import Spec, Src0, Src1, C0, C1, C2, maxx, minn

    v = Src0 + C0 * Src1
    body = v - minn(maxx(v, C2), C1)

    def _ref(in0, in1, s0, s1, imm2):
        v = in0.astype(np.float32) + np.float32(s0) * in1.astype(np.float32)
        return v - np.minimum(np.maximum(v, np.float32(imm2)), np.float32(s1))

    return _register_dve_op("FISTA_SHRINK_ANT", Spec(body=body, reference=_ref))


def _get_shrink0_op():
    """out = in0 - clamp(in0, -s1, s1)  (softshrink only; used at iteration 0
    where x_0 = 0 so there is no momentum term)."""
    from concourse.dve_spec import Spec, Src0, C0, C1, maxx, minn

    body = Src0 - minn(maxx(Src0, C0), C1)

    def _ref(in0, in1, s0, s1, imm2):
        v = in0.astype(np.float32)
        return v - np.minimum(np.maximum(v, np.float32(s0)), np.float32(s1))

    return _register_dve_op("FISTA_SHRINK0_ANT", Spec(body=body, reference=_ref))


# --------------------------------------------------------------------------- #
# host-side precompute
# --------------------------------------------------------------------------- #
def _build_dictionary(rr, theta, t):
    i = np.arange(t, dtype=np.float64)[:, None]
    rr = rr.astype(np.float64)
    theta = theta.astype(np.float64)
    rp = rr[None, :] ** i
    sgn = np.where(np.arange(t)[:, None] % 2 == 0, 1.0, -1.0)
    c = np.cos(i * theta[None, :])
    s = np.sin(i * theta[None, :])
    ones = np.ones((t, 1))
    dic = np.concatenate([ones, rp * c, sgn * rp * c, rp * s, sgn * rp * s], axis=1)
    g = np.linalg.norm(dic, axis=0)
    g = np.where(g == 0, np.sqrt(t), g)
    return dic / g


def _momentum_coeffs(n_iter):
    ts = []
    t = 1.0
    for _ in range(n_iter):
        t_new = (1.0 + np.sqrt(1.0 + 4.0 * t * t)) / 2.0
        ts.append((t - 1.0) / t_new)
        t = t_new
    return np.asarray(ts, dtype=np.float32)


# --------------------------------------------------------------------------- #
# device module
# --------------------------------------------------------------------------- #
def _build_module(lam, linv, tts):
    import concourse.bacc as bacc
    import concourse.mybir as mybir
    import concourse.tile as tile

    F32 = mybir.dt.float32
    F32R = mybir.dt.float32r
    F16 = mybir.dt.float16
    BF16 = mybir.dt.bfloat16
    shrink_op = _get_shrink_op()
    shrink0_op = _get_shrink0_op()

    nc = bacc.Bacc("TRN2", target_bir_lowering=False, debug=False)

    y_d = nc.dram_tensor("y_in", [T, P], F32, kind="ExternalInput").ap()
    sd_d = nc.dram_tensor("s_d", [K + T, 128], BF16, kind="ExternalInput").ap()
    wab_d = nc.dram_tensor("w_ab", [42, 768], F16, kind="ExternalInput").ap()
    wab2_d = nc.dram_tensor("w_ab2", [42, 768], F16, kind="ExternalInput").ap()
    z_d = nc.dram_tensor("zeros", [22, P], F16, kind="ExternalInput").ap()
    z32_d = nc.dram_tensor("zeros32", [CH[5], P], F32, kind="ExternalInput").ap()
    out_d = nc.dram_tensor("out", [K, P], F32, kind="ExternalOutput").ap()

    # per-iteration scalars (fp32-exact python floats)
    tt_prev = [0.0] + [float(tts[j]) for j in range(NITER - 1)]
    lam_f = float(np.float32(lam))
    linv_f = float(np.float32(linv))

    with tile.TileContext(nc) as tc:
        with (
            tc.tile_pool(name="const", bufs=1) as const,
            tc.tile_pool(name="state", bufs=1) as state,
            tc.tile_pool(name="upool", bufs=2, space="PSUM") as upool,
            tc.tile_pool(name="wpool", bufs=3, space="PSUM") as wpool,
        ):
            wab_t = const.tile([42, 768], F16, tag="wab", name="wab_t")
            # h1's w stationary lives at partitions 64:106 (row groups 2-3)
            wab2_t = const.tile([106, 768], F16, tag="wab2", name="wab2_t")
            sd_t = [const.tile([XR[c], 128], BF16, tag=f"sd{c}", name=f"sd_t{c}")
                    for c in range(6)]

            nc.sync.dma_start(out=wab_t[:], in_=wab_d[:])
            nc.sync.dma_start(out=wab2_t[64:106, :], in_=wab2_d[:])
            so = [0, 128, 256, 384, 512, 640, 655]
            for c in range(6):
                nc.sync.dma_start(out=sd_t[c][:], in_=sd_d[so[c]:so[c + 1], :])

            xt = [[state.tile([XR[c], P], F32, tag=f"x{g}_{c}", name=f"x{g}_{c}")
                   for c in range(6)] for g in range(3)]
            # The two pixel halves run in disjoint PE-array quadrant sets:
            # h0 occupies partitions 0:42, h1 partitions 64:106, so each u
            # pair (col groups 0-1 vs 2-3) and each w pair (row groups 0-1 vs
            # 2-3, different psum banks) executes concurrently (tile_position).
            ab_ts = [state.tile([42, P], F16, tag=f"AB{p}", name=f"ab_t{p}")
                     for p in range(2)]
            ab2_ts = [state.tile([106, P], F16, tag=f"AB2{p}", name=f"ab2_t{p}")
                      for p in range(2)]
            # rows 10..31 (74..95 of the h1 mirror) are dead contraction lanes
            # of the merged matmul: must be finite (stationary rows are zero)
            for p in range(2):
                nc.sync.dma_start(out=ab_ts[p][10:32, :], in_=z_d[:])
                nc.sync.dma_start(out=ab2_ts[p][74:96, :], in_=z_d[:])
            # chunk 5 carries Y in rows 5:15 of every generation; rows 0:5 of
            # the j=0 generation must be zero (x_0 = 0)
            for g in range(3):
                nc.sync.dma_start(out=xt[g][5][5:15, :], in_=y_d[:])
            nc.sync.dma_start(out=xt[0][5][0:5, :], in_=z32_d[:])

            def bfview(ap):
                return ap.bitcast(BF16)[:, 1::2]

            # Iteration specialization:
            #   j=0: x_0 = x_{-1} = 0 -> u_0 = Y via the chunk-5 matmul alone,
            #        no pre-write, plain shrink, w group start=True.
            #   j=1: tt_prev = 0 -> no pre-write, w start=True; B_0 is zero
            #        data but the b-term matmul contraction (42 rows) is used.
            for j in range(NITER):
                ttp = tt_prev[j]
                gm1, g0, g1 = (j + 2) % 3, j % 3, (j + 1) % 3
                ab_cur = ab_ts[j % 2]
                ab_next = ab_ts[(j + 1) % 2]
                ab2_cur = ab2_ts[j % 2]
                ab2_next = ab2_ts[(j + 1) % 2]
                a_scale = float(np.float32((1.0 + ttp) * linv_f))
                b_scale = float(np.float32(-float(tts[j]) * linv_f))
                has_ux = j >= 1        # x_j nonzero
                has_id = ttp != 0.0    # j >= 2
                has_m2old = j >= 1     # B_{j-1} exists

                # u = Y - D x_j  (replicated at partition offsets 0/32; Y rides
                # the chunk-5 stationary's identity rows). Both pixel halves
                # share ONE psum bank: h0 at partitions 0:42 (col groups 0-1),
                # h1 at 64:106 (col groups 2-3) — the pair runs concurrently.
                cs_u = list(range(6)) if has_ux else [5]
                u_ps = upool.tile([128, NH], F32, tag="u", name="u_ps")
                for ci, c in enumerate(cs_u):
                    xv = bfview(xt[g0][c][:])
                    nc.tensor.matmul(u_ps[0:42, :], sd_t[c][:, 0:42],
                                     xv[:, 0:NH],
                                     start=(ci == 0), stop=(c == 5),
                                     skip_group_check=True,
                                     tile_position=(0, 0))
                    nc.tensor.matmul(u_ps[64:106, :], sd_t[c][:, 0:42],
                                     xv[:, NH:P],
                                     start=(ci == 0), stop=(c == 5),
                                     skip_group_check=True,
                                     tile_position=(0, 64))
                # scaled copies: A_j = (1+tt)/L u_j (used now),
                #                B_j = -tts[j]/L u_j (used next iteration)
                nc.scalar.mul(ab_cur[0:T, 0:NH], u_ps[0:T, :], a_scale)
                nc.scalar.mul(ab2_cur[64:74, NH:P], u_ps[64:74, :], a_scale)
                if j < NITER - 1:
                    nc.scalar.mul(ab_next[32:42, 0:NH], u_ps[32:42, :],
                                  b_scale)
                    nc.scalar.mul(ab2_next[96:106, NH:P], u_ps[96:106, :],
                                  b_scale)

                for wv in (0, 1):
                    cs = [3 * wv, 3 * wv + 1, 3 * wv + 2]
                    wt = {c: wpool.tile([CH[c], P], F32, tag="w", name=f"w{c}")
                          for c in cs}
                    # identity part: w = -tt * x_{j-1} as an exact fp32
                    # engine pre-write; the matmuls accumulate onto it
                    # (has_written bits persist from the j<2 start=True groups)
                    if has_id:
                        for c in cs:
                            src = xt[gm1][c][0:CH[c], :]
                            if PRE_ENG[c] == "act":
                                nc.scalar.mul(wt[c][:], src, float(np.float32(-ttp)))
                            else:
                                nc.vector.tensor_scalar_mul(wt[c][:], src,
                                                            float(np.float32(-ttp)))
                    # rank-10+10 part in one matmul: w += [D;0;D]^T [A;junk;B]
                    # h0 (row groups 0-1) and h1 (row groups 2-3) write the
                    # two psum banks of wt[c] concurrently
                    kc = 42 if has_m2old else T
                    for c in cs:
                        cw = slice(128 * c, 128 * c + CH[c])
                        nc.tensor.matmul(
                            wt[c][:, 0:NH],
                            wab_t[0:kc, cw],
                            ab_cur[0:kc, 0:NH],
                            start=not has_id, stop=True,
                            skip_group_check=True,
                            tile_position=(0, 0))
                        nc.tensor.matmul(
                            wt[c][:, NH:P],
                            wab2_t[64:64 + kc, cw],
                            ab2_cur[64:64 + kc, NH:P],
                            start=not has_id, stop=True,
                            skip_group_check=True,
                            tile_position=(64, 0))
                    # x_{j+1} = shrink(w + (1+tt) x_j)
                    for c in cs:
                        if has_ux:
                            nc.vector._custom_dve(
                                shrink_op, out=xt[g1][c][0:CH[c], :],
                                in0=wt[c][:],
                                in1=xt[g0][c][0:CH[c], :],
                                s0=float(np.float32(1.0 + ttp)), s1=lam_f,
                                imm2=-lam_f)
                        else:
                            nc.vector._custom_dve(
                                shrink0_op, out=xt[g1][c][0:CH[c], :],
                                in0=wt[c][:],
                                s0=-lam_f, s1=lam_f)
                        if j == NITER - 1:
                            nc.sync.dma_start(
                                out=out_d[OFF[c]:OFF[c] + CH[c], :],
                                in_=xt[g1][c][0:CH[c], :])

    nc.compile()
    return nc


# --------------------------------------------------------------------------- #
# entry point
# --------------------------------------------------------------------------- #
def _prepare(x, Drr, Dtheta, t):
    import ml_dtypes

    x = np.asarray(x, dtype=np.float32)
    d64 = _build_dictionary(np.asarray(Drr), np.asarray(Dtheta), t)
    dtd = d64.T @ d64
    lspec = np.linalg.norm(dtd, ord=2)
    linv = 1.0 / lspec
    lam = LAMBD * linv
    d32 = d64.astype(np.float32)
    tts = _momentum_coeffs(NITER)

    # u = Y - D x is produced replicated at partition offsets 0 and 32 (the
    # 0-copy feeds the A scaled-copy, the 32-copy feeds the B scaled-copy).
    # Stationary free dims padded to 128 so FWL (fast weight load) triggers.
    # Rows K:K+T are the chunk-5 identity rows that inject Y.
    s_d = np.zeros((K + T, 128), dtype=np.float32)
    for r in (0, 1):
        s_d[:K, 32 * r:32 * r + T] = -d32.T
        s_d[K + np.arange(T), 32 * r + np.arange(T)] = 1.0
    # merged rank-20 stationary: rows 0..9 multiply A, rows 32..41 multiply B
    w_ab = np.zeros((42, 768), dtype=np.float32)
    for c in range(6):
        w_ab[0:T, 128 * c:128 * c + CH[c]] = d32[:, OFF[c]:OFF[c] + CH[c]]
        w_ab[32:42, 128 * c:128 * c + CH[c]] = d32[:, OFF[c]:OFF[c] + CH[c]]
    zeros = np.zeros((22, P), dtype=np.float16)
    zeros32 = np.zeros((CH[5], P), dtype=np.float32)
    return (x, lam, linv, tts, s_d.astype(ml_dtypes.bfloat16),
            w_ab.astype(np.float16), zeros, zeros32)


def run(x, Drr, Dtheta, T_in, trace=False):
    from concourse.bass_utils import run_bass_kernel_spmd

    t = int(np.asarray(T_in))
    assert t == T
    x, lam, linv, tts, s_d, w_ab, zeros, zeros32 = _prepare(x, Drr, Dtheta, t)

    key = ("mod", float(np.float32(lam)), float(np.float32(linv)))
    if key not in _cache:
        _cache[key] = _build_module(lam, linv, tts)
    nc = _cache[key]

    in_maps = []
    for core in range(N_CORES):
        in_maps.append({
            "y_in": np.ascontiguousarray(x[0, :, core * P:(core + 1) * P]),
            "s_d": s_d,
            "w_ab": w_ab,
            "w_ab2": w_ab,
            "zeros": zeros,
            "zeros32": zeros32,
        })
    res = run_bass_kernel_spmd(nc, in_maps, list(range(N_CORES)), trace=trace)
    out = np.concatenate([res.results[c]["out"] for c in range(N_CORES)], axis=1)
    return out[None, :, :].astype(np.float32), res


def kernel(x, Drr, Dtheta, T, **kw):
    out, _ = run(x, Drr, Dtheta, T, trace=bool(os.environ.get("FISTA_TRACE")))
    return out
